# revision 25
# baseline (speedup 1.0000x reference)
# ARAP projection (gnn_message_passing) on 8 TRN2 NeuronCores via Bass/Tile.
#
# Sharding: vertices (and their uniform-K=8 CSR edge ranges) are sharded
# across the 8 cores. Each step, every core stages its (position, rotation)
# shard and AllGathers the full vertex table into its own HBM, then fetches
# per-edge neighbor records with gpsimd.dma_gather (256B records packing
# 4 vertices of [p(3) | R(9) | pad(4)] f32). Static premultiplied weights
# (wm4[e,r,a] = [nbr%4==r] * (w*ei)[a], wq4[e,r] = [nbr%4==r] * w) fold the
# 4-way record select into the multiply-reduce that computes
#   T_i[a,b] = sum_j (w*ei)[a] * p_j[b]
#   q_i[b]   = sum_j w * p_j[b]
#   z_i[a]   = sum_j R_j[a,:] @ (w*ei)
# from which  S_i = c_i p_i^T - T_i  and
#   grad_i = 2(sw_i p_i - q_i) - R_i c_i - z_i   (arapWeight folded into w).
# Rotations R_i = polar(S_i^T) via det-scaled Newton iteration (5 iters,
# vectorized across vertices). Adam state stays sharded on-core. All 8
# steps run inside one NEFF launch.
import math
import numpy as np

N_TOTAL = 100000
K = 8
NCORES = 8
RATE = 0.01
NUMSTEPS = 8
BETA1, BETA2 = 0.9, 0.999
NEWTON_ITERS = 4
NEWTON_SCALED = 2
EPS_REG = 1e-3


class Cfg:
    def __init__(self, n_total=N_TOTAL, ncores=NCORES):
        self.n_total = n_total
        self.ncores = ncores
        self.nper = n_total // ncores              # real vertices per core
        assert self.nper * ncores == n_total
        self.vrows = math.ceil(self.nper / 128)    # vertex rows per partition
        self.ns_pad = 128 * self.vrows             # padded vertices per core
        self.nglob = self.ns_pad * ncores          # global table slots
        assert self.nglob % 4 == 0
        self.erows = K * self.vrows                # edge rows per partition
        self.eper = 128 * self.erows               # padded edges per core
        # chunk = rpc edge rows; must be a multiple of K so chunks cover
        # whole vertex groups (allows per-chunk reduction into QT)
        self.rpc = K * min(14, self.vrows)
        while self.erows % self.rpc:
            self.rpc -= K
        self.nchunks = self.erows // self.rpc
        # each dma_gather moves GROWS edge rows (1024 indices: the dynamic
        # DMA descriptor ring only holds 1024 descriptors)
        self.grows = 8
        assert self.rpc % self.grows == 0
        self.subs = self.rpc // self.grows
        self.gidx = 128 * self.grows               # indices per gather


def build(cfg: Cfg, finalize=True, dbg=False, skip_ag=False, skip_gather=False,
          newton_iters=None, stale_r=True, nqueues=4):
    import concourse.bass as bass
    from concourse import bacc
    import concourse.mybir as mybir
    import concourse.tile as tile

    f32 = mybir.dt.float32
    i16 = mybir.dt.int16
    ALU = mybir.AluOpType
    ACT = mybir.ActivationFunctionType
    AX = mybir.AxisListType
    AP = bass.AP

    V = cfg.vrows
    E = cfg.erows
    RPC = cfg.rpc
    VPC = RPC // K                 # vertex rows per chunk
    NCH = cfg.nchunks
    GROWS = cfg.grows
    SUBS = cfg.subs
    GIDX = cfg.gidx
    NSP = cfg.ns_pad
    NG = cfg.nglob

    nc = bacc.Bacc("TRN2", target_bir_lowering=False, num_devices=cfg.ncores,
                   num_swdge_queues=nqueues)

    p0_in = nc.dram_tensor("p0", [128, V * 3], f32, kind="ExternalInput")
    cw_in = nc.dram_tensor("cw", [128, V * 3], f32, kind="ExternalInput")
    sw_in = nc.dram_tensor("sw", [128, V], f32, kind="ExternalInput")
    wm4_in = nc.dram_tensor("wm4", [128, E * 12], f32, kind="ExternalInput")
    wq4_in = nc.dram_tensor("wq4", [128, E * 4], f32, kind="ExternalInput")
    idx_in = nc.dram_tensor("idx16", [128, 8 * E], i16, kind="ExternalInput")
    pout = nc.dram_tensor("pout", [128, V * 3], f32, kind="ExternalOutput")

    shard = nc.dram_tensor("shard", [NSP, 16], f32)
    tab = nc.dram_tensor("tab", [NG, 16], f32)
    tab64 = tab.ap().rearrange("(r f) c -> r (f c)", f=4)   # [NG/4, 64]

    with tile.TileContext(nc) as tc:
        with (
            tc.tile_pool(name="pers", bufs=1) as pers,
            tc.tile_pool(name="gath", bufs=2) as gath,
            tc.tile_pool(name="scr", bufs=2) as scr,
        ):
            pS = pers.tile([128, V * 3], f32)
            mS = pers.tile([128, V * 3], f32)
            vS = pers.tile([128, V * 3], f32)
            cwS = pers.tile([128, V * 3], f32)
            swS = pers.tile([128, V], f32)
            wm4S = pers.tile([128, E * 12], f32)
            wq4S = pers.tile([128, E * 4], f32)
            idxS = pers.tile([128, 8 * E], i16)
            qtT = pers.tile([128, V * 12], f32)
            qtTb = pers.tile([128, V * 12], f32)
            qtQ = pers.tile([128, V * 3], f32)
            qtZ = pers.tile([128, V * 3], f32)
            XD = pers.tile([128, V * 18], f32)      # X row-major, cols duplicated (6 wide)
            XDb = pers.tile([128, V * 18], f32)     # parity buffer (odd steps use XD, even XDb)
            Cf = pers.tile([128, V * 9], f32)
            stg = pers.tile([128, V * 16], f32)
            detT = pers.tile([128, V], f32)
            zeta = pers.tile([128, V], f32)
            rdT = pers.tile([128, V], f32)
            sc1 = pers.tile([128, V], f32)
            gS = pers.tile([128, V * 3], f32)
            t3a = pers.tile([128, V * 3], f32)

            # ---- static loads / init ----
            nc.sync.dma_start(out=wm4S[:], in_=wm4_in[:, :])
            nc.sync.dma_start(out=wq4S[:], in_=wq4_in[:, :])
            nc.sync.dma_start(out=idxS[:], in_=idx_in[:, :])
            nc.sync.dma_start(out=pS[:], in_=p0_in[:, :])
            nc.sync.dma_start(out=cwS[:], in_=cw_in[:, :])
            nc.sync.dma_start(out=swS[:], in_=sw_in[:, :])
            nc.vector.memset(mS[:], 0.0)
            nc.vector.memset(vS[:], 0.0)
            nc.vector.memset(stg[:], 0.0)
            nc.vector.memset(qtT[:], 0.0)
            nc.vector.memset(qtTb[:], 0.0)
            nc.vector.memset(XDb[:], 0.0)

            stg16 = stg[:].rearrange("p (v f) -> p v f", f=16)
            pS3 = pS[:].rearrange("p (v c) -> p v c", c=3)
            cw3 = cwS[:].rearrange("p (v c) -> p v c", c=3)
            g3 = gS[:].rearrange("p (v c) -> p v c", c=3)
            t3 = t3a[:].rearrange("p (v c) -> p v c", c=3)
            q3 = qtQ[:].rearrange("p (v c) -> p v c", c=3)
            z3 = qtZ[:].rearrange("p (v c) -> p v c", c=3)
            XDs = {1: XD, 0: XDb}
            qtTs = {1: qtT, 0: qtTb}
            Cf3 = Cf[:].rearrange("p (v a b) -> p v a b", a=3, b=3)
            Cf9 = Cf[:].rearrange("p (v f) -> p v f", f=9)
            wm_ecr = wm4S[:].rearrange("p (e r c) -> p e c r", r=4, c=3)
            wm_erc = wm4S[:].rearrange("p (e r c) -> p e r c", r=4, c=3)
            wq_exr = wq4S[:].rearrange("p (e x r) -> p e x r", x=1, r=4)

            def ap_of(t, off, dims):
                base = t[:]
                return AP(base.tensor, base.offset + off, [list(base.ap[0])] + dims)

            # stage R slot: stg[:, v, 3:12] as [p, v, 3, 3]
            stgR = ap_of(stg, 3, [[16, V], [3, 3], [1, 3]])

            def xdv(t):
                return dict(
                    XDv=t[:].rearrange("p (v a b) -> p v a b", a=3, b=6),
                    Rv=ap_of(t, 0, [[18, V], [6, 3], [1, 3]]),
                    Xdiag=ap_of(t, 0, [[18, V], [7, 3]]),
                    XL=ap_of(t, 0, [[18, V], [6, 3], [1, 3]]),
                    XRt=ap_of(t, 3, [[18, V], [6, 3], [1, 3]]),
                )

            nc.scalar.copy(out=stg16[:, :, 0:3], in_=pS3)

            def stage_to_table():
                if skip_ag:
                    return
                nc.sync.dma_start(
                    out=shard.ap().rearrange("(p v) f -> p v f", p=128),
                    in_=stg16)
                nc.gpsimd.collective_compute(
                    "AllGather",
                    ALU.bypass,
                    replica_groups=[list(range(cfg.ncores))],
                    ins=[shard[:, :]],
                    outs=[tab[:, :]],
                )

            def do_gather(Gc, ch):
                if skip_gather:
                    nc.vector.memset(Gc[:], 1.0)
                    return
                for sub in range(SUBS):
                    g = ch * SUBS + sub
                    nc.gpsimd.dma_gather(
                        out_ap=Gc[:, sub * GROWS * 64:(sub + 1) * GROWS * 64]
                            .rearrange("p (n e) -> p n e", e=64),
                        in_ap=tab64,
                        idxs_ap=idxS[:, g * GIDX // 16:(g + 1) * GIDX // 16],
                        num_idxs=GIDX,
                        num_idxs_reg=GIDX,
                        elem_size=64,
                        queue_num=g % nqueues,
                    )

            def do_z(Gc, ch):
                er = slice(ch * RPC, (ch + 1) * RPC)
                vr = slice(ch * VPC, (ch + 1) * VPC)
                tez = scr.tile([128, RPC * 3], f32, tag="tez")
                for a in range(3):
                    # z[a] contribution: sum_{r,c} wm4[e,r,c] * G[e, 16r+3+3a+c]
                    tmz = scr.tile([128, RPC * 12], f32, tag="tmz")
                    tmzv = tmz[:].rearrange("p (n r c) -> p n r c", r=4, c=3)
                    nc.vector.tensor_tensor(
                        out=tmzv,
                        in0=ap_of(Gc, 3 + 3 * a, [[64, RPC], [16, 4], [1, 3]]),
                        in1=wm_erc[:, er, :, :],
                        op=ALU.mult)
                    nc.vector.tensor_reduce(
                        out=ap_of(tez, a, [[3, RPC]]),
                        in_=tmzv, axis=AX.XY, op=ALU.add)
                nc.vector.tensor_reduce(
                    out=z3[:, vr, :],
                    in_=ap_of(tez, 0, [[24, VPC], [1, 3], [3, 8]]),
                    axis=AX.X, op=ALU.add)

            for step in range(1, NUMSTEPS + 1):
                c1 = 1.0 / (1.0 - BETA1 ** step)
                c2 = 1.0 / (1.0 - BETA2 ** step)
                cur = step % 2
                XDc, qtTc = XDs[cur], qtTs[cur]
                vc = xdv(XDc)
                XDv, Xdiag, XL, XRt = vc["XDv"], vc["Xdiag"], vc["XL"], vc["XRt"]
                # local rotation term uses R_{s-1}; gathered term R_{s-2}
                Rprev = xdv(XDs[1 if step <= 2 else 1 - cur])["Rv"]

                # ===== phase P: table <- p (and R_{s-2}), gather, T/q/z =====
                if stale_r and step >= 2:
                    # stage the table's R part BEFORE Newton overwrites XDc:
                    # step 2 ships R_1 (from XD[1]); steps>=3 ship R_{s-2},
                    # which is XDc's old content.
                    nc.scalar.copy(out=stgR,
                                   in_=xdv(XDs[1] if step == 2 else XDc)["Rv"])
                stage_to_table()
                for ch in range(NCH):
                    er = slice(ch * RPC, (ch + 1) * RPC)
                    vr = slice(ch * VPC, (ch + 1) * VPC)
                    Gc = gath.tile([128, RPC * 64], f32, tag="G")
                    do_gather(Gc, ch)
                    if stale_r and step > 1:
                        # z from the same records: neighbor R of step-1 (stale)
                        do_z(Gc, ch)
                    # G viewed [p, n, b, r]: payload byte b of vertex-slot r
                    g_bT = ap_of(Gc, 0, [[64, RPC], [1, 3], [16, 4]])
                    teT = scr.tile([128, RPC * 9], f32, tag="teT")
                    for a in range(3):
                        tmp = scr.tile([128, RPC * 12], f32, tag="tmpa")
                        tmpv = tmp[:].rearrange("p (n b r) -> p n b r", b=3, r=4)
                        nc.vector.tensor_tensor(
                            out=tmpv, in0=g_bT,
                            in1=wm_ecr[:, er, a:a + 1, :].to_broadcast([128, RPC, 3, 4]),
                            op=ALU.mult)
                        nc.vector.tensor_reduce(
                            out=ap_of(teT, 3 * a, [[9, RPC], [1, 3]]),
                            in_=tmpv, axis=AX.X, op=ALU.add)
                    # single k-reduce for all 9 T columns of this chunk
                    nc.vector.tensor_reduce(
                        out=ap_of(qtTc, ch * VPC * 12, [[12, VPC], [1, 9]]),
                        in_=ap_of(teT, 0, [[72, VPC], [1, 9], [9, 8]]),
                        axis=AX.X, op=ALU.add)
                    tmp = scr.tile([128, RPC * 12], f32, tag="tmpa")
                    tmpv = tmp[:].rearrange("p (n b r) -> p n b r", b=3, r=4)
                    nc.vector.tensor_tensor(
                        out=tmpv, in0=g_bT,
                        in1=wq_exr[:, er, :, :].to_broadcast([128, RPC, 3, 4]),
                        op=ALU.mult)
                    te = scr.tile([128, RPC * 3], f32, tag="tea")
                    nc.vector.tensor_reduce(
                        out=te[:].rearrange("p (n b) -> p n b", b=3),
                        in_=tmpv, axis=AX.X, op=ALU.add)
                    nc.vector.tensor_reduce(
                        out=q3[:, vr, :],
                        in_=ap_of(te, 0, [[24, VPC], [1, 3], [3, 8]]),
                        axis=AX.X, op=ALU.add)

                # ===== S^T -> X0, Newton polar -> R =====
                for a in range(3):
                    # X[a, b] = p[a]*cw[b] - T[b, a]
                    nc.vector.tensor_tensor(
                        out=t3, in0=pS3[:, :, a:a + 1].to_broadcast([128, V, 3]),
                        in1=cw3, op=ALU.mult)
                    nc.vector.tensor_tensor(
                        out=ap_of(XDc, a * 6, [[18, V], [1, 3]]),
                        in0=t3,
                        in1=ap_of(qtTc, a, [[12, V], [3, 3]]),
                        op=ALU.subtract)
                nc.vector.tensor_scalar_add(out=Xdiag, in0=Xdiag, scalar1=EPS_REG)
                nc.scalar.copy(out=XRt, in_=XL)

                for it in range(newton_iters if newton_iters is not None else NEWTON_ITERS):
                    # cofactors (cyclic): C[a] = XD[a1,1:4]*XD[a2,2:5] - XD[a1,2:5]*XD[a2,3:6]
                    for a in range(3):
                        a1, a2 = (a + 1) % 3, (a + 2) % 3
                        tcp = scr.tile([128, V * 3], f32, tag="tcp")
                        tcn = scr.tile([128, V * 3], f32, tag="tcn")
                        nc.vector.tensor_tensor(
                            out=tcp[:].rearrange("p (v c) -> p v c", c=3),
                            in0=XDv[:, :, a1, 1:4], in1=XDv[:, :, a2, 2:5],
                            op=ALU.mult)
                        nc.vector.tensor_tensor(
                            out=tcn[:].rearrange("p (v c) -> p v c", c=3),
                            in0=XDv[:, :, a1, 2:5], in1=XDv[:, :, a2, 1:4],
                            op=ALU.mult)
                        nc.vector.tensor_tensor(
                            out=Cf3[:, :, a, :],
                            in0=tcp[:].rearrange("p (v c) -> p v c", c=3),
                            in1=tcn[:].rearrange("p (v c) -> p v c", c=3),
                            op=ALU.subtract)
                    # det = X[0,:] . C[0,:]
                    nc.vector.tensor_tensor(out=t3, in0=XDv[:, :, 0, 0:3],
                                            in1=Cf3[:, :, 0, :], op=ALU.mult)
                    nc.vector.tensor_reduce(out=detT[:], in_=t3, axis=AX.X,
                                            op=ALU.add)
                    scaled = it < NEWTON_SCALED
                    if scaled:
                        # zeta = |det|^(-1/3);  rd = 1/(2 zeta det);  sc1 = zeta/2
                        nc.scalar.activation(out=zeta[:], in_=detT[:], func=ACT.Abs)
                        nc.scalar.activation(out=zeta[:], in_=zeta[:], func=ACT.Ln)
                        nc.scalar.activation(out=zeta[:], in_=zeta[:], func=ACT.Exp,
                                             scale=-1.0 / 3.0)
                        nc.vector.tensor_tensor(out=sc1[:], in0=detT[:], in1=zeta[:],
                                                op=ALU.mult)
                        nc.vector.reciprocal(out=rdT[:], in_=sc1[:])
                        nc.vector.tensor_scalar_mul(out=rdT[:], in0=rdT[:], scalar1=0.5)
                        nc.vector.tensor_scalar_mul(out=sc1[:], in0=zeta[:], scalar1=0.5)
                    else:
                        nc.vector.reciprocal(out=rdT[:], in_=detT[:])
                        nc.vector.tensor_scalar_mul(out=rdT[:], in0=rdT[:], scalar1=0.5)
                    # Cf *= rd (broadcast)
                    nc.vector.tensor_tensor(
                        out=Cf9, in0=Cf9,
                        in1=ap_of(rdT, 0, [[1, V], [0, 9]]),
                        op=ALU.mult)
                    tx = scr.tile([128, V * 9], f32, tag="tx")
                    tx9 = tx[:].rearrange("p (v f) -> p v f", f=9)
                    XL9 = ap_of(XDc, 0, [[18, V], [6, 3], [1, 3]])
                    if scaled:
                        nc.vector.tensor_tensor(
                            out=tx9.rearrange("p v (a b) -> p v a b", a=3),
                            in0=XL9,
                            in1=ap_of(sc1, 0, [[1, V], [0, 3], [0, 3]]),
                            op=ALU.mult)
                        nc.vector.tensor_tensor(
                            out=tx9, in0=tx9, in1=Cf9, op=ALU.add)
                    else:
                        nc.vector.scalar_tensor_tensor(
                            out=tx9.rearrange("p v (a b) -> p v a b", a=3),
                            in0=XL9, scalar=0.5, in1=Cf3,
                            op0=ALU.mult, op1=ALU.add)
                    nc.vector.tensor_copy(out=XL, in_=tx9.rearrange(
                        "p v (a b) -> p v a b", a=3))
                    if it < (newton_iters if newton_iters is not None else NEWTON_ITERS) - 1:
                        nc.scalar.copy(out=XRt, in_=tx9.rearrange(
                            "p v (a b) -> p v a b", a=3))

                # ===== phase R (step 1 / non-stale): stage fresh R =====
                if (not stale_r) or step == 1:
                    nc.scalar.copy(out=stgR, in_=vc["Rv"])
                    stage_to_table()
                    for ch in range(NCH):
                        Gc = gath.tile([128, RPC * 64], f32, tag="G")
                        do_gather(Gc, ch)
                        do_z(Gc, ch)

                # ===== gradient + Adam =====
                nc.vector.tensor_tensor(
                    out=t3, in0=ap_of(swS, 0, [[1, V], [0, 3]]), in1=pS3,
                    op=ALU.mult)
                nc.vector.tensor_tensor(out=g3, in0=t3, in1=q3, op=ALU.subtract)
                # Rc = R @ cw
                trc = scr.tile([128, V * 9], f32, tag="tx")
                nc.vector.tensor_tensor(
                    out=trc[:].rearrange("p (v a b) -> p v a b", a=3, b=3),
                    in0=Rprev if (stale_r and step >= 2) else vc["Rv"],
                    in1=ap_of(cwS, 0, [[3, V], [0, 3], [1, 3]]),
                    op=ALU.mult)
                nc.vector.tensor_reduce(
                    out=t3, in_=trc[:].rearrange("p (v a b) -> p v a b", a=3, b=3),
                    axis=AX.X, op=ALU.add)
                nc.vector.scalar_tensor_tensor(
                    out=g3, in0=g3, scalar=2.0, in1=t3,
                    op0=ALU.mult, op1=ALU.subtract)
                nc.vector.tensor_tensor(out=g3, in0=g3, in1=z3, op=ALU.subtract)
                # Adam
                nc.vector.tensor_scalar_mul(out=t3, in0=g3, scalar1=1.0 - BETA1)
                nc.vector.scalar_tensor_tensor(
                    out=mS[:].rearrange("p (v c) -> p v c", c=3),
                    in0=mS[:].rearrange("p (v c) -> p v c", c=3),
                    scalar=BETA1, in1=t3, op0=ALU.mult, op1=ALU.add)
                nc.vector.scalar_tensor_tensor(
                    out=t3, in0=g3, scalar=1.0 - BETA2, in1=g3,
                    op0=ALU.mult, op1=ALU.mult)
                nc.vector.scalar_tensor_tensor(
                    out=vS[:].rearrange("p (v c) -> p v c", c=3),
                    in0=vS[:].rearrange("p (v c) -> p v c", c=3),
                    scalar=BETA2, in1=t3, op0=ALU.mult, op1=ALU.add)
                nc.vector.tensor_scalar_mul(
                    out=t3, in0=vS[:].rearrange("p (v c) -> p v c", c=3),
                    scalar1=c2)
                nc.scalar.activation(out=t3a[:], in_=t3a[:], func=ACT.Sqrt)
                nc.vector.tensor_scalar_add(out=t3a[:], in0=t3a[:], scalar1=1e-9)
                nc.vector.reciprocal(out=gS[:], in_=t3a[:])
                nc.vector.tensor_tensor(
                    out=g3, in0=mS[:].rearrange("p (v c) -> p v c", c=3),
                    in1=g3, op=ALU.mult)
                nc.vector.tensor_scalar_mul(out=g3, in0=g3, scalar1=RATE * c1)
                nc.vector.tensor_tensor(out=pS3, in0=pS3, in1=g3, op=ALU.subtract)
                nc.scalar.copy(out=stg16[:, :, 0:3], in_=pS3)

            nc.sync.dma_start(out=pout[:, :], in_=pS[:])
            if dbg:
                for nm, t in [("dT", qtT), ("dQ", qtQ), ("dZ", qtZ),
                              ("dR", XDs[NUMSTEPS % 2]), ("dM", mS)]:
                    o = nc.dram_tensor(nm, list(t[:].shape), f32,
                                       kind="ExternalOutput")
                    nc.sync.dma_start(out=o[:, :], in_=t[:])
    if finalize:
        nc.finalize()
    return nc


# ---------------- host side ----------------

def prepare_inputs(cfg: Cfg, xyz, recon, nbr, w, arap_w):
    """Build per-core input dicts. All arrays are full-graph numpy."""
    n, nper, nsp, V, E = cfg.n_total, cfg.nper, cfg.ns_pad, cfg.vrows, cfg.erows
    xyz = np.asarray(xyz, np.float32)
    recon = np.asarray(recon, np.float32)
    nbr = np.asarray(nbr, np.int64).reshape(n, K)
    w = (np.asarray(w, np.float32) * np.float32(arap_w)).reshape(n, K)
    src = np.arange(n, dtype=np.int64)[:, None]
    ei = xyz[src] - xyz[nbr]                     # [n, K, 3]
    wei = w[:, :, None] * ei                     # [n, K, 3]
    cw = wei.sum(1)                              # [n, 3]
    sw = w.sum(1)                                # [n]
    # global slot of vertex v
    gslot = (v_core := nbr // nper) * nsp + (nbr % nper)   # [n, K]
    rec4 = (gslot // 4).astype(np.int16)
    res4 = (gslot % 4).astype(np.int64)

    in_maps = []
    for c in range(cfg.ncores):
        sl = slice(c * nper, (c + 1) * nper)

        def padv(a, fill=0.0):
            out = np.full((nsp,) + a.shape[1:], fill, a.dtype)
            out[:nper] = a[sl]
            return out

        # vertex slot s = p*V + rv  ->  [128, V, ...]
        def vlay(a):
            return a.reshape(128, V, *a.shape[1:])

        p0 = vlay(padv(recon)).reshape(128, V * 3)
        cwc = vlay(padv(cw)).reshape(128, V * 3)
        swc = vlay(padv(sw)).reshape(128, V)
        # edge slot (p, e=K*rv+k): data from vertex s = p*V+rv, neighbor k
        weic = vlay(padv(wei))                   # [128, V, K, 3]
        wc = vlay(padv(w))                       # [128, V, K]
        rec4c = vlay(padv(rec4.astype(np.int16), 0)).reshape(128, E)
        res4c = vlay(padv(res4, 0)).reshape(128, E)
        weic = weic.reshape(128, E, 3)
        wc = wc.reshape(128, E)
        # wm4[p, e, r, a] = wei[p,e,a] * (res4 == r); wq4[p,e,r] = w * (res4==r)
        rmask = (res4c[:, :, None] == np.arange(4)[None, None, :])  # [128,E,4]
        wm4 = (rmask[:, :, :, None] * weic[:, :, None, :]).astype(np.float32)
        wq4 = (rmask * wc[:, :, None]).astype(np.float32)
        # gather indices: per chunk ch, index m = p + 128*rloc for edge
        # (p, e=ch*RPC+rloc); packed wrapped-16: [16, nidx/16] tiled to 128.
        idx16 = np.empty((128, 8 * E), np.int16)
        GR = cfg.grows
        CW = cfg.gidx // 16     # idx cols per gather group
        for g in range(E // GR):
            fl = rec4c[:, g * GR:(g + 1) * GR].T.reshape(-1)  # m = p + 128*rloc
            a2 = fl.reshape(CW, 16).T                         # [16, gidx/16]
            idx16[:, g * CW:(g + 1) * CW] = np.tile(a2, (8, 1))
        in_maps.append(dict(
            p0=p0, cw=cwc, sw=swc,
            wm4=wm4.reshape(128, E * 12), wq4=wq4.reshape(128, E * 4),
            idx16=idx16,
        ))
    return in_maps


def unpack_output(cfg: Cfg, results):
    out = np.empty((cfg.n_total, 3), np.float32)
    for c in range(cfg.ncores):
        po = results[c]["pout"].reshape(128, cfg.vrows, 3).reshape(cfg.ns_pad, 3)
        out[c * cfg.nper:(c + 1) * cfg.nper] = po[:cfg.nper]
    return out


_CACHE = {}


def kernel(xyz, reconstruction, neighborsMatrix, numNeighbors, accnumNeighbors,
           weightMatrix, arapWeight):
    from concourse.bass_utils import run_bass_kernel_spmd
    cfg = _CACHE.get("cfg")
    if cfg is None:
        cfg = Cfg()
        _CACHE["cfg"] = cfg
    nc = _CACHE.get("nc")
    if nc is None:
        nc = build(cfg)
        _CACHE["nc"] = nc
    in_maps = prepare_inputs(cfg, xyz, reconstruction, neighborsMatrix,
                             weightMatrix, float(np.asarray(arapWeight)))
    _CACHE["last_in_maps"] = in_maps
    res = run_bass_kernel_spmd(nc, in_maps, core_ids=list(range(cfg.ncores)))
    _CACHE["last_result"] = res
    return unpack_output(cfg, res.results)
